# revision 13
# baseline (speedup 1.0000x reference)
"""DeformConv1d Trainium2 Bass kernel.

Problem: x[4,512,4096] f32, offsets[4,1,4090,7] f32, weight[512,512,7], bias[512]
  T[b,o,k]   = clamp(o + k + offsets[b,0,o,k], o, o+6)
  samp[b,c,o,k] = linear-interp of x[b,c,:] at T
  out[b,d,o] = sum_{c,k} samp[b,c,o,k] * weight[d,c,k] + bias[d]

Key identity: the clamp keeps every sample inside [o, o+6], so output o only
touches taps p in [o, o+7], and the interpolation weight of tap p is exactly
relu(1 - |p - T|).  With an o-tile of 121 the tap band is exactly 128 wide:

  out[o,d] = sum_{k, dp<128} S_k[dp, o] * Y[o0+dp, k, d] + bias[d]
    S_k[dp,o] = relu(1 - |(c_k[o] + (o-o0)) - dp|), c_k[o] = clamp(k+off, 0, 6)
    Y[p,k,d]  = sum_c x[c, p] * weight[d, c, k]

Both stages are dense bf16 matmuls on the PE array (f32 PSUM accumulate).
PE LDWEIGHTS dual-buffering hides all weight loads at the 512-col stream
rate, so the schedule is tuned around DMA arrival and queue ordering:
 - a single DMA ring serializes its transfers at ~110GB/s (and has only
   ~4 recyclable semaphores, so a tiny transfer with ~3us completion
   latency at the head blocks the chunks behind it), while rings overlap;
   the bulk loads are split in HALVES over the three DMA-capable rings
   (sync/scalar/gpsimd), round-robined in exact PE consumption order so
   every k-slice of W lands just as the interleaved stage-1 k-loops of
   tiles 0/1 need it, and the late x chunks ride behind;
 - the S pipeline is nearly DMA-free and diag-free: the HOST precomputes
   the per-tile row layout of c+j where c = clamp(k+off,0,6) (68KB
   input); each tile loads its 4KB row, expands it across partitions
   with the GPSIMD partition_broadcast instruction, subtracts the
   partition index (a [128,1] tensor_scalar), |x| on DVE, relu on ACT;
   Pool runs ONLY partition_broadcast so exactly one GPSIMD ucode
   library is ever loaded (a Pool tensor op from another library would
   force a ~12us library swap per tile);
 - interpolation weights of each (o,k) sum to exactly 1, so bias is
   folded into the k=0 Y eviction (tensor_tensor add of a pre-broadcast
   bias tile) -- no bias matmul;
 - output is stored bf16 and cast/transposed on the host.
Sharding: 8 cores = 4 batches x 2 halves of out_len (data parallel, no comm).
"""

import os
import sys

import ml_dtypes
import numpy as np

for _p in ("/opt/trn_rl_repo", os.path.expanduser("~/.axon_site/_ro/trn_rl_repo")):
    if os.path.isdir(_p) and _p not in sys.path:
        sys.path.insert(0, _p)

import concourse.mybir as mybir
import concourse.tile as tile
from concourse import bacc
from concourse.bass_utils import run_bass_kernel_spmd

B, CIN, COUT, L, K = 4, 512, 512, 4096, 7
OUT_LEN = 4090
HALF = 2045          # out positions per core (2 halves per batch)
OT = 121             # o-tile size -> tap band = OT + 7 = 128
TILES = 17           # 17 * 121 = 2057 >= 2045
OPAD = TILES * OT    # 2057 padded out positions per core
XW = (TILES - 1) * OT + 128  # 2064: rightmost x column any tile reads
P = 128
NCK = CIN // P       # 4 c-chunks
# x chunks: [0:256) covers tiles 0/1, [256:384) tile 2, then bulk
XCH = ((0, 256), (256, 384), (384, 644), (644, 1368), (1368, XW))
F32 = mybir.dt.float32
BF16 = mybir.dt.bfloat16

_prog_cache = {}


def _build_program():
    nc = bacc.Bacc("TRN2", target_bir_lowering=False, debug=False)

    xs_ds = [
        nc.dram_tensor(f"xs{i}", [P, NCK, hi - lo], BF16, kind="ExternalInput")
        for i, (lo, hi) in enumerate(XCH)
    ]
    wt_d = nc.dram_tensor("wt", [P, K, NCK, COUT], BF16, kind="ExternalInput")
    c2_d = nc.dram_tensor("c2", [TILES, 1024], F32, kind="ExternalInput")
    biasf_d = nc.dram_tensor("biasf", [1, COUT], F32, kind="ExternalInput")
    dp_d = nc.dram_tensor("dpcol", [P, 1], F32, kind="ExternalInput")
    out_d = nc.dram_tensor("out", [OPAD, COUT], BF16, kind="ExternalOutput")

    with tile.TileContext(nc) as tc:
        with (
            tc.tile_pool(name="const", bufs=1) as cpool,
            tc.tile_pool(name="cbt", bufs=4) as cbpool,
            tc.tile_pool(name="crows", bufs=4) as crpool,
            tc.tile_pool(name="stiles", bufs=4) as stpool,
            tc.tile_pool(name="ytiles", bufs=3) as ypool,
            tc.tile_pool(name="otiles", bufs=3) as opool,
            tc.tile_pool(name="psy", bufs=5, space="PSUM") as psy,
            tc.tile_pool(name="pso", bufs=3, space="PSUM") as pso,
        ):
            xs = cpool.tile([P, NCK, XW], BF16)
            wt = cpool.tile([P, K, NCK, COUT], BF16)
            biasf = cpool.tile([1, COUT], F32)
            dpcol = cpool.tile([P, 1], F32)
            bias_bc = cpool.tile([P, COUT], F32)

            def xs_load(ring, i, cl, ch):
                lo, hi = XCH[i]
                ring.dma_start(xs[:, cl:ch, lo:hi], xs_ds[i][:, cl:ch, :])

            # ---- bulk loads: halves round-robined over the 3 rings in PE
            # consumption order (xs0, then wt k0..k6, then late x chunks).
            # sync ring:
            xs_load(nc.sync, 0, 0, 2)                            # xs0 ci01
            for k, half in ((0, (2, 4)), (2, (0, 2)), (3, (2, 4)),
                            (5, (0, 2)), (6, (2, 4))):
                nc.sync.dma_start(wt[:, k, half[0] : half[1]],
                                  wt_d[:, k, half[0] : half[1]])
            xs_load(nc.sync, 3, 0, NCK)                          # xs[644:1368)
            xs_load(nc.sync, 4, 0, NCK)                          # xs[1368:)
            # scalar ring:
            for k, half in ((0, (0, 2)), (1, (0, 2)), (2, (2, 4)),
                            (4, (0, 2))):
                nc.scalar.dma_start(wt[:, k, half[0] : half[1]],
                                    wt_d[:, k, half[0] : half[1]])
            xs_load(nc.scalar, 1, 0, NCK)                        # xs[256:384)
            nc.scalar.dma_start(wt[:, 5, 2:4], wt_d[:, 5, 2:4])
            xs_load(nc.scalar, 2, 0, NCK)                        # xs[384:644)
            # gpsimd ring (also runs the per-tile c-row pipeline below):
            nc.gpsimd.dma_start(biasf[:], biasf_d[:])
            nc.gpsimd.dma_start(dpcol[:], dp_d[:])
            xs_load(nc.gpsimd, 0, 2, 4)                          # xs0 ci23
            for k, half in ((1, (2, 4)), (3, (0, 2)), (4, (2, 4)),
                            (6, (0, 2))):
                nc.gpsimd.dma_start(wt[:, k, half[0] : half[1]],
                                    wt_d[:, k, half[0] : half[1]])
            # bias row -> all partitions, on-chip (for the k=0 bias fold)
            nc.gpsimd.partition_broadcast(bias_bc[:], biasf[:])

            reg = slice(0, 512 + 3 * OT)

            def s_build(t):
                """load 4KB (c+j)-row, expand across partitions ON-CHIP,
                then s = relu(1 - |row - dp|): pbcast on Pool, sub+|x| on
                DVE, relu on ACT."""
                crow = crpool.tile([1, 1024], F32, tag="crow", name=f"cr{t}")
                cb = cbpool.tile([P, 1024], F32, tag="cb", name=f"cb{t}")
                s_sb = stpool.tile([P, 1024], BF16, tag="s_sb", name=f"s{t}")
                nc.gpsimd.dma_start(crow[:], c2_d[t : t + 1, :])
                nc.gpsimd.partition_broadcast(cb[:, reg], crow[:, reg])
                nc.vector.tensor_scalar(
                    cb[:, reg], cb[:, reg], dpcol[:], None,
                    mybir.AluOpType.subtract,
                )
                nc.vector.scalar_tensor_tensor(
                    cb[:, reg], cb[:, reg], -1.0, cb[:, reg],
                    mybir.AluOpType.mult, mybir.AluOpType.max,
                )
                nc.scalar.activation(
                    s_sb[:, reg], cb[:, reg],
                    mybir.ActivationFunctionType.Relu,
                    bias=1.0, scale=-1.0,
                )
                return s_sb

            def y_group(y_sb, o0, k):
                """Y_k for band [o0, o0+128) -> y_sb[:, k, :], bias on k=0."""
                yp = psy.tile([P, COUT], F32, tag="yp", name="yp")
                for ci in range(NCK):
                    nc.tensor.matmul(
                        yp[:],
                        xs[:, ci, o0 : o0 + P],
                        wt[:, k, ci, :],
                        start=(ci == 0), stop=(ci == NCK - 1),
                    )
                if k == 0:
                    # S_k columns sum to exactly 1 => folding bias into Y_0
                    # adds bias[d] to out[o, d] once
                    nc.vector.tensor_tensor(
                        y_sb[:, 0, :], yp[:], bias_bc[:], mybir.AluOpType.add,
                    )
                elif k < 3:
                    nc.vector.tensor_copy(y_sb[:, k, :], yp[:])
                else:
                    nc.scalar.copy(y_sb[:, k, :], yp[:])

            def stage2(s_sb, y_sb, o0, t):
                """out[o, d] = sum_k S_k^T Y_k, evict bf16, store."""
                op = pso.tile([P, COUT], F32, tag="op", name="op")
                for k in range(K):
                    koff = k * OT if k < 4 else 512 + (k - 4) * OT
                    nc.tensor.matmul(
                        op[:OT],
                        s_sb[:, koff : koff + OT],
                        y_sb[:, k, :],
                        start=(k == 0), stop=(k == K - 1),
                    )
                o_sb = opool.tile([P, COUT], BF16, tag="o_sb", name="o_sb")
                if t % 2 == 0:
                    nc.scalar.copy(o_sb[:OT], op[:OT])
                else:
                    nc.vector.tensor_copy(o_sb[:OT], op[:OT])
                nc.sync.dma_start(out_d[o0 : o0 + OT, :], o_sb[:OT])

            # ---- tiles 0,1: stage-1 k-loops interleaved so PE consumption
            # (2 x 4 matmuls per k) rides the per-k weight arrival rate
            s0, s1 = s_build(0), s_build(1)
            y0 = ypool.tile([P, K, COUT], BF16, tag="y_sb", name="y0")
            y1 = ypool.tile([P, K, COUT], BF16, tag="y_sb", name="y1")
            for k in range(K):
                y_group(y0, 0, k)
                y_group(y1, OT, k)
            stage2(s0, y0, 0, 0)
            stage2(s1, y1, OT, 1)

            # ---- tiles 2..16, steady state ----
            for t in range(2, TILES):
                o0 = t * OT
                s_sb = s_build(t)
                y_sb = ypool.tile([P, K, COUT], BF16, tag="y_sb", name="y_sb")
                for k in range(K):
                    y_group(y_sb, o0, k)
                stage2(s_sb, y_sb, o0, t)

    nc.compile()
    return nc


def _install_axon_ntff_hook():
    """Provide antenv.axon_hooks (absent on this image) so that
    run_bass_kernel_spmd(trace=True) can capture NTFF profiles via the
    axon .so's C ABI.  Mirrors trn_agent_boot.trn_boot."""
    import contextlib
    import ctypes
    import types

    try:
        from antenv.axon_hooks import set_axon_ntff_profile_hook  # noqa: F401
        return
    except ImportError:
        pass

    so_path = "/opt/axon/libaxon_pjrt.so"
    if not os.path.exists(so_path):
        return
    lib = ctypes.CDLL(so_path)
    if not hasattr(lib, "axon_start_nrt_profile"):
        return
    lib.axon_start_nrt_profile.argtypes = [
        ctypes.POINTER(ctypes.c_int64), ctypes.c_size_t,
    ]
    lib.axon_start_nrt_profile.restype = ctypes.c_int64
    lib.axon_stop_nrt_profile.argtypes = [ctypes.c_char_p]
    lib.axon_stop_nrt_profile.restype = ctypes.c_int64

    @contextlib.contextmanager
    def _hook(output_dir, device_ids):
        import jax

        jax.devices()
        if device_ids:
            ids = (ctypes.c_int64 * len(device_ids))(*device_ids)
            rc = lib.axon_start_nrt_profile(ids, len(device_ids))
        else:
            rc = lib.axon_start_nrt_profile(None, 0)
        if rc != 0:
            raise RuntimeError(f"axon_start_nrt_profile rc={rc}")
        try:
            yield
        finally:
            n = lib.axon_stop_nrt_profile(str(output_dir).encode())
            print(f"ntff profile: {n} file(s) written to {output_dir}")

    box = {"h": _hook}
    mod = types.ModuleType("antenv.axon_hooks")
    mod.get_axon_ntff_profile_hook = lambda: box["h"]
    mod.set_axon_ntff_profile_hook = lambda h: box.__setitem__("h", h)
    import antenv

    sys.modules["antenv.axon_hooks"] = mod
    antenv.axon_hooks = mod

    # zero-egress env: skip the artifact upload in the trace path
    from concourse import bass_utils as _bu

    _bu.upload_artifacts = lambda d: f"local:{d}"


def kernel(x, offsets, weight, bias, _trace=False, _trace_kwargs=None):
    x = np.asarray(x, dtype=np.float32)
    offsets = np.asarray(offsets, dtype=np.float32)
    weight = np.asarray(weight, dtype=np.float32)
    bias = np.asarray(bias, dtype=np.float32)

    if "nc" not in _prog_cache:
        _prog_cache["nc"] = _build_program()
    nc = _prog_cache["nc"]

    # W packed [p, k, ci, d] so each per-k DMA is contiguous per partition
    w4 = np.ascontiguousarray(
        np.transpose(weight, (1, 2, 0))              # [c, k, d]
        .reshape(NCK, P, K, COUT)
        .transpose(1, 2, 0, 3)                       # [p, k, ci, d]
        .astype(ml_dtypes.bfloat16)
    )
    biasf = np.ascontiguousarray(bias.reshape(1, COUT).astype(np.float32))
    dpcol = np.arange(P, dtype=np.float32).reshape(P, 1).copy()
    karr = np.arange(K, dtype=np.float32).reshape(K, 1)
    # jrow[koff(k)+j] = j, matching the bank-padded S layout
    jrow = np.zeros(1024, dtype=np.float32)
    for i in range(4):
        jrow[i * OT : (i + 1) * OT] = np.arange(OT)
        if i < 3:
            jrow[512 + i * OT : 512 + (i + 1) * OT] = np.arange(OT)

    in_maps = []
    for core in range(8):
        b, half = core // 2, core % 2
        o_off = half * HALF
        xsf = np.zeros((P, NCK, XW), dtype=ml_dtypes.bfloat16)
        xw = min(L - o_off, XW)
        # x rows c = ci*128 + p -> [p, ci, t]
        xsf[:, :, :xw] = (
            x[b][:, o_off : o_off + xw]
            .reshape(NCK, P, xw)
            .transpose(1, 0, 2)
            .astype(ml_dtypes.bfloat16)
        )
        # host-side S prep: c+j = clamp(k + off, 0, 6) + j, relayouted into
        # per-tile rows (k<4 blocks at 0, k>=4 blocks at 512, each OT wide)
        offsT = np.zeros((K, OPAD), dtype=np.float32)
        ow = min(OUT_LEN - o_off, OPAD)
        offsT[:, :ow] = offsets[b, 0, o_off : o_off + ow, :].T
        cfull = np.clip(karr + offsT, 0.0, 6.0)         # [K, OPAD]
        cblk = cfull.reshape(K, TILES, OT)              # [K, T, OT]
        c2 = np.zeros((TILES, 1024), dtype=np.float32)
        c2[:, 0 : 4 * OT] = (
            cblk[0:4].transpose(1, 0, 2).reshape(TILES, 4 * OT)
        )
        c2[:, 512 : 512 + 3 * OT] = (
            cblk[4:7].transpose(1, 0, 2).reshape(TILES, 3 * OT)
        )
        c2 += jrow[None, :]
        im = {"wt": w4, "c2": c2, "biasf": biasf, "dpcol": dpcol}
        for i, (lo, hi) in enumerate(XCH):
            im[f"xs{i}"] = np.ascontiguousarray(xsf[:, :, lo:hi])
        in_maps.append(im)

    if _trace:
        _install_axon_ntff_hook()
    try:
        res = run_bass_kernel_spmd(
            nc, in_maps, core_ids=list(range(8)),
            trace=_trace, **(_trace_kwargs or {}),
        )
    except Exception:
        # transient runtime faults have been observed; one retry
        res = run_bass_kernel_spmd(
            nc, in_maps, core_ids=list(range(8)),
            trace=_trace, **(_trace_kwargs or {}),
        )

    out = np.empty((B, COUT, OUT_LEN), dtype=np.float32)
    for core in range(8):
        b, half = core // 2, core % 2
        o_off = half * HALF
        out[b, :, o_off : o_off + HALF] = (
            res.results[core]["out"][:HALF, :].astype(np.float32).T
        )
    if _trace:
        _prog_cache["last_exec_time_ns"] = res.exec_time_ns
    return out


# revision 14
# speedup vs baseline: 1.0040x; 1.0040x over previous
"""DeformConv1d Trainium2 Bass kernel.

Problem: x[4,512,4096] f32, offsets[4,1,4090,7] f32, weight[512,512,7], bias[512]
  T[b,o,k]   = clamp(o + k + offsets[b,0,o,k], o, o+6)
  samp[b,c,o,k] = linear-interp of x[b,c,:] at T
  out[b,d,o] = sum_{c,k} samp[b,c,o,k] * weight[d,c,k] + bias[d]

Key identity: the clamp keeps every sample inside [o, o+6], so output o only
touches taps p in [o, o+7], and the interpolation weight of tap p is exactly
relu(1 - |p - T|).  With an o-tile of 121 the tap band is exactly 128 wide:

  out[o,d] = sum_{k, dp<128} S_k[dp, o] * Y[o0+dp, k, d] + bias[d]
    S_k[dp,o] = relu(1 - |(c_k[o] + (o-o0)) - dp|), c_k[o] = clamp(k+off, 0, 6)
    Y[p,k,d]  = sum_c x[c, p] * weight[d, c, k]

Both stages are dense bf16 matmuls on the PE array (f32 PSUM accumulate).
PE LDWEIGHTS dual-buffering hides all weight loads at the 512-col stream
rate, so the schedule is tuned around DMA arrival and queue ordering:
 - a single DMA ring serializes its transfers at ~110GB/s (and has only
   ~4 recyclable semaphores, so a tiny transfer with ~3us completion
   latency at the head blocks the chunks behind it), while rings overlap;
   the bulk loads are split in HALVES over the three DMA-capable rings
   (sync/scalar/gpsimd), round-robined in exact PE consumption order so
   every k-slice of W lands just as the interleaved stage-1 k-loops of
   tiles 0/1 need it, and the late x chunks ride behind;
 - the S pipeline is nearly DMA-free and diag-free: the HOST precomputes
   the per-tile row layout of c+j where c = clamp(k+off,0,6) (68KB
   input); each tile loads its 4KB row, expands it across partitions
   with the GPSIMD partition_broadcast instruction, subtracts the
   partition index (a [128,1] tensor_scalar), |x| on DVE, relu on ACT;
   Pool runs ONLY partition_broadcast so exactly one GPSIMD ucode
   library is ever loaded (a Pool tensor op from another library would
   force a ~12us library swap per tile);
 - interpolation weights of each (o,k) sum to exactly 1, so bias is
   folded into the k=0 Y eviction (tensor_tensor add of a pre-broadcast
   bias tile) -- no bias matmul;
 - output is stored bf16 and cast/transposed on the host.
Sharding: 8 cores = 4 batches x 2 halves of out_len (data parallel, no comm).
"""

import os
import sys

import ml_dtypes
import numpy as np

for _p in ("/opt/trn_rl_repo", os.path.expanduser("~/.axon_site/_ro/trn_rl_repo")):
    if os.path.isdir(_p) and _p not in sys.path:
        sys.path.insert(0, _p)

import concourse.mybir as mybir
import concourse.tile as tile
from concourse import bacc
from concourse.bass_utils import run_bass_kernel_spmd

B, CIN, COUT, L, K = 4, 512, 512, 4096, 7
OUT_LEN = 4090
HALF = 2045          # out positions per core (2 halves per batch)
OT = 121             # o-tile size -> tap band = OT + 7 = 128
TILES = 17           # 17 * 121 = 2057 >= 2045
OPAD = TILES * OT    # 2057 padded out positions per core
XW = (TILES - 1) * OT + 128  # 2064: rightmost x column any tile reads
P = 128
NCK = CIN // P       # 4 c-chunks
# x chunks: [0:256) covers tiles 0/1, [256:384) tile 2, then bulk
XCH = ((0, 256), (256, 384), (384, 644), (644, 1368), (1368, XW))
F32 = mybir.dt.float32
BF16 = mybir.dt.bfloat16

_prog_cache = {}


def _build_program():
    nc = bacc.Bacc("TRN2", target_bir_lowering=False, debug=False)

    xs_ds = [
        nc.dram_tensor(f"xs{i}", [P, NCK, hi - lo], BF16, kind="ExternalInput")
        for i, (lo, hi) in enumerate(XCH)
    ]
    wt_d = nc.dram_tensor("wt", [P, K, NCK, COUT], BF16, kind="ExternalInput")
    c2_d = nc.dram_tensor("c2", [TILES, 1024], F32, kind="ExternalInput")
    biasf_d = nc.dram_tensor("biasf", [1, COUT], F32, kind="ExternalInput")
    dp_d = nc.dram_tensor("dpcol", [P, 1], F32, kind="ExternalInput")
    out_d = nc.dram_tensor("out", [OPAD, COUT], BF16, kind="ExternalOutput")

    with tile.TileContext(nc) as tc:
        with (
            tc.tile_pool(name="const", bufs=1) as cpool,
            tc.tile_pool(name="cbt", bufs=4) as cbpool,
            tc.tile_pool(name="crows", bufs=4) as crpool,
            tc.tile_pool(name="stiles", bufs=4) as stpool,
            tc.tile_pool(name="ytiles", bufs=3) as ypool,
            tc.tile_pool(name="otiles", bufs=3) as opool,
            tc.tile_pool(name="psy", bufs=5, space="PSUM") as psy,
            tc.tile_pool(name="pso", bufs=3, space="PSUM") as pso,
        ):
            xs = cpool.tile([P, NCK, XW], BF16)
            wt = cpool.tile([P, K, NCK, COUT], BF16)
            biasf = cpool.tile([1, COUT], F32)
            dpcol = cpool.tile([P, 1], F32)
            bias_bc = cpool.tile([P, COUT], F32)

            def xs_load(ring, i, cl, ch):
                lo, hi = XCH[i]
                ring.dma_start(xs[:, cl:ch, lo:hi], xs_ds[i][:, cl:ch, :])

            # ---- bulk loads: per-k halves on two clean rings in PE
            # consumption order, x band + tinies on the gpsimd ring.
            # sync ring: first halves of every k, then the late x chunks
            for k in range(K):
                nc.sync.dma_start(wt[:, k, 0:2], wt_d[:, k, 0:2])
            xs_load(nc.sync, 3, 0, NCK)                          # xs[644:1368)
            xs_load(nc.sync, 4, 0, NCK)                          # xs[1368:)
            # scalar ring: second halves of every k, then mid x chunks
            for k in range(K):
                nc.scalar.dma_start(wt[:, k, 2:4], wt_d[:, k, 2:4])
            xs_load(nc.scalar, 1, 0, NCK)                        # xs[256:384)
            xs_load(nc.scalar, 2, 0, NCK)                        # xs[384:644)
            # gpsimd ring (also runs the per-tile c-row pipeline below):
            xs_load(nc.gpsimd, 0, 0, NCK)                        # xs0 whole
            nc.gpsimd.dma_start(biasf[:], biasf_d[:])
            nc.gpsimd.dma_start(dpcol[:], dp_d[:])
            # bias row -> all partitions, on-chip (for the k=0 bias fold)
            nc.gpsimd.partition_broadcast(bias_bc[:], biasf[:])

            reg = slice(0, 512 + 3 * OT)

            def s_build(t):
                """load 4KB (c+j)-row, expand across partitions ON-CHIP,
                then s = relu(1 - |row - dp|): pbcast on Pool, sub+|x| on
                DVE, relu on ACT."""
                crow = crpool.tile([1, 1024], F32, tag="crow", name=f"cr{t}")
                cb = cbpool.tile([P, 1024], F32, tag="cb", name=f"cb{t}")
                s_sb = stpool.tile([P, 1024], BF16, tag="s_sb", name=f"s{t}")
                nc.gpsimd.dma_start(crow[:], c2_d[t : t + 1, :])
                nc.gpsimd.partition_broadcast(cb[:, reg], crow[:, reg])
                nc.vector.tensor_scalar(
                    cb[:, reg], cb[:, reg], dpcol[:], None,
                    mybir.AluOpType.subtract,
                )
                nc.vector.scalar_tensor_tensor(
                    cb[:, reg], cb[:, reg], -1.0, cb[:, reg],
                    mybir.AluOpType.mult, mybir.AluOpType.max,
                )
                nc.scalar.activation(
                    s_sb[:, reg], cb[:, reg],
                    mybir.ActivationFunctionType.Relu,
                    bias=1.0, scale=-1.0,
                )
                return s_sb

            def y_group(y_sb, o0, k):
                """Y_k for band [o0, o0+128) -> y_sb[:, k, :], bias on k=0."""
                yp = psy.tile([P, COUT], F32, tag="yp", name="yp")
                for ci in range(NCK):
                    nc.tensor.matmul(
                        yp[:],
                        xs[:, ci, o0 : o0 + P],
                        wt[:, k, ci, :],
                        start=(ci == 0), stop=(ci == NCK - 1),
                    )
                if k == 0:
                    # S_k columns sum to exactly 1 => folding bias into Y_0
                    # adds bias[d] to out[o, d] once
                    nc.vector.tensor_tensor(
                        y_sb[:, 0, :], yp[:], bias_bc[:], mybir.AluOpType.add,
                    )
                elif k < 3:
                    nc.vector.tensor_copy(y_sb[:, k, :], yp[:])
                else:
                    nc.scalar.copy(y_sb[:, k, :], yp[:])

            def stage2(s_sb, y_sb, o0, t):
                """out[o, d] = sum_k S_k^T Y_k, evict bf16, store."""
                op = pso.tile([P, COUT], F32, tag="op", name="op")
                for k in range(K):
                    koff = k * OT if k < 4 else 512 + (k - 4) * OT
                    nc.tensor.matmul(
                        op[:OT],
                        s_sb[:, koff : koff + OT],
                        y_sb[:, k, :],
                        start=(k == 0), stop=(k == K - 1),
                    )
                o_sb = opool.tile([P, COUT], BF16, tag="o_sb", name="o_sb")
                if t % 2 == 0:
                    nc.scalar.copy(o_sb[:OT], op[:OT])
                else:
                    nc.vector.tensor_copy(o_sb[:OT], op[:OT])
                nc.sync.dma_start(out_d[o0 : o0 + OT, :], o_sb[:OT])

            # ---- tiles 0,1: stage-1 k-loops interleaved so PE consumption
            # (2 x 4 matmuls per k) rides the per-k weight arrival rate
            s0, s1 = s_build(0), s_build(1)
            y0 = ypool.tile([P, K, COUT], BF16, tag="y_sb", name="y0")
            y1 = ypool.tile([P, K, COUT], BF16, tag="y_sb", name="y1")
            for k in range(K):
                y_group(y0, 0, k)
                y_group(y1, OT, k)
            stage2(s0, y0, 0, 0)
            stage2(s1, y1, OT, 1)

            # ---- tiles 2..16, steady state ----
            for t in range(2, TILES):
                o0 = t * OT
                s_sb = s_build(t)
                y_sb = ypool.tile([P, K, COUT], BF16, tag="y_sb", name="y_sb")
                for k in range(K):
                    y_group(y_sb, o0, k)
                stage2(s_sb, y_sb, o0, t)

    nc.compile()
    return nc


def _install_axon_ntff_hook():
    """Provide antenv.axon_hooks (absent on this image) so that
    run_bass_kernel_spmd(trace=True) can capture NTFF profiles via the
    axon .so's C ABI.  Mirrors trn_agent_boot.trn_boot."""
    import contextlib
    import ctypes
    import types

    try:
        from antenv.axon_hooks import set_axon_ntff_profile_hook  # noqa: F401
        return
    except ImportError:
        pass

    so_path = "/opt/axon/libaxon_pjrt.so"
    if not os.path.exists(so_path):
        return
    lib = ctypes.CDLL(so_path)
    if not hasattr(lib, "axon_start_nrt_profile"):
        return
    lib.axon_start_nrt_profile.argtypes = [
        ctypes.POINTER(ctypes.c_int64), ctypes.c_size_t,
    ]
    lib.axon_start_nrt_profile.restype = ctypes.c_int64
    lib.axon_stop_nrt_profile.argtypes = [ctypes.c_char_p]
    lib.axon_stop_nrt_profile.restype = ctypes.c_int64

    @contextlib.contextmanager
    def _hook(output_dir, device_ids):
        import jax

        jax.devices()
        if device_ids:
            ids = (ctypes.c_int64 * len(device_ids))(*device_ids)
            rc = lib.axon_start_nrt_profile(ids, len(device_ids))
        else:
            rc = lib.axon_start_nrt_profile(None, 0)
        if rc != 0:
            raise RuntimeError(f"axon_start_nrt_profile rc={rc}")
        try:
            yield
        finally:
            n = lib.axon_stop_nrt_profile(str(output_dir).encode())
            print(f"ntff profile: {n} file(s) written to {output_dir}")

    box = {"h": _hook}
    mod = types.ModuleType("antenv.axon_hooks")
    mod.get_axon_ntff_profile_hook = lambda: box["h"]
    mod.set_axon_ntff_profile_hook = lambda h: box.__setitem__("h", h)
    import antenv

    sys.modules["antenv.axon_hooks"] = mod
    antenv.axon_hooks = mod

    # zero-egress env: skip the artifact upload in the trace path
    from concourse import bass_utils as _bu

    _bu.upload_artifacts = lambda d: f"local:{d}"


def kernel(x, offsets, weight, bias, _trace=False, _trace_kwargs=None):
    x = np.asarray(x, dtype=np.float32)
    offsets = np.asarray(offsets, dtype=np.float32)
    weight = np.asarray(weight, dtype=np.float32)
    bias = np.asarray(bias, dtype=np.float32)

    if "nc" not in _prog_cache:
        _prog_cache["nc"] = _build_program()
    nc = _prog_cache["nc"]

    # W packed [p, k, ci, d] so each per-k DMA is contiguous per partition
    w4 = np.ascontiguousarray(
        np.transpose(weight, (1, 2, 0))              # [c, k, d]
        .reshape(NCK, P, K, COUT)
        .transpose(1, 2, 0, 3)                       # [p, k, ci, d]
        .astype(ml_dtypes.bfloat16)
    )
    biasf = np.ascontiguousarray(bias.reshape(1, COUT).astype(np.float32))
    dpcol = np.arange(P, dtype=np.float32).reshape(P, 1).copy()
    karr = np.arange(K, dtype=np.float32).reshape(K, 1)
    # jrow[koff(k)+j] = j, matching the bank-padded S layout
    jrow = np.zeros(1024, dtype=np.float32)
    for i in range(4):
        jrow[i * OT : (i + 1) * OT] = np.arange(OT)
        if i < 3:
            jrow[512 + i * OT : 512 + (i + 1) * OT] = np.arange(OT)

    in_maps = []
    for core in range(8):
        b, half = core // 2, core % 2
        o_off = half * HALF
        xsf = np.zeros((P, NCK, XW), dtype=ml_dtypes.bfloat16)
        xw = min(L - o_off, XW)
        # x rows c = ci*128 + p -> [p, ci, t]
        xsf[:, :, :xw] = (
            x[b][:, o_off : o_off + xw]
            .reshape(NCK, P, xw)
            .transpose(1, 0, 2)
            .astype(ml_dtypes.bfloat16)
        )
        # host-side S prep: c+j = clamp(k + off, 0, 6) + j, relayouted into
        # per-tile rows (k<4 blocks at 0, k>=4 blocks at 512, each OT wide)
        offsT = np.zeros((K, OPAD), dtype=np.float32)
        ow = min(OUT_LEN - o_off, OPAD)
        offsT[:, :ow] = offsets[b, 0, o_off : o_off + ow, :].T
        cfull = np.clip(karr + offsT, 0.0, 6.0)         # [K, OPAD]
        cblk = cfull.reshape(K, TILES, OT)              # [K, T, OT]
        c2 = np.zeros((TILES, 1024), dtype=np.float32)
        c2[:, 0 : 4 * OT] = (
            cblk[0:4].transpose(1, 0, 2).reshape(TILES, 4 * OT)
        )
        c2[:, 512 : 512 + 3 * OT] = (
            cblk[4:7].transpose(1, 0, 2).reshape(TILES, 3 * OT)
        )
        c2 += jrow[None, :]
        im = {"wt": w4, "c2": c2, "biasf": biasf, "dpcol": dpcol}
        for i, (lo, hi) in enumerate(XCH):
            im[f"xs{i}"] = np.ascontiguousarray(xsf[:, :, lo:hi])
        in_maps.append(im)

    if _trace:
        _install_axon_ntff_hook()
    try:
        res = run_bass_kernel_spmd(
            nc, in_maps, core_ids=list(range(8)),
            trace=_trace, **(_trace_kwargs or {}),
        )
    except Exception:
        # transient runtime faults have been observed; one retry
        res = run_bass_kernel_spmd(
            nc, in_maps, core_ids=list(range(8)),
            trace=_trace, **(_trace_kwargs or {}),
        )

    out = np.empty((B, COUT, OUT_LEN), dtype=np.float32)
    for core in range(8):
        b, half = core // 2, core % 2
        o_off = half * HALF
        out[b, :, o_off : o_off + HALF] = (
            res.results[core]["out"][:HALF, :].astype(np.float32).T
        )
    if _trace:
        _prog_cache["last_exec_time_ns"] = res.exec_time_ns
    return out


# revision 15
# speedup vs baseline: 1.0379x; 1.0337x over previous
"""DeformConv1d Trainium2 Bass kernel.

Problem: x[4,512,4096] f32, offsets[4,1,4090,7] f32, weight[512,512,7], bias[512]
  T[b,o,k]   = clamp(o + k + offsets[b,0,o,k], o, o+6)
  samp[b,c,o,k] = linear-interp of x[b,c,:] at T
  out[b,d,o] = sum_{c,k} samp[b,c,o,k] * weight[d,c,k] + bias[d]

Key identity: the clamp keeps every sample inside [o, o+6], so output o only
touches taps p in [o, o+7], and the interpolation weight of tap p is exactly
relu(1 - |p - T|).  With an o-tile of 121 the tap band is exactly 128 wide:

  out[o,d] = sum_{k, dp<128} S_k[dp, o] * Y[o0+dp, k, d] + bias[d]
    S_k[dp,o] = relu(1 - |(dp - (o-o0)) - c_k[o]|),  c_k[o] = clamp(k + off, 0, 6)
    Y[p,k,d]  = sum_c x[c, p] * weight[d, c, k]

Both stages are dense bf16 matmuls on the PE array (f32 PSUM accumulate).
PE LDWEIGHTS dual-buffering hides all weight loads at the 512-col stream
rate, so the schedule is tuned around DMA arrival and queue ordering:
 - the host pre-packs W as [p, k, ci, d] and x as per-chunk contiguous
   blocks so every bulk DMA moves 2KB+ contiguous per partition;
 - a single DMA ring serializes its transfers at ~110GB/s while rings run
   in parallel, so the startup-critical loads are SPLIT ACROSS the three
   DMA-capable rings (sync/scalar/gpsimd) in PE consumption order; tiles
   0/1 run their stage-1 k-loops INTERLEAVED to ride the arrival rate;
 - the S pipeline is nearly DMA-free: c = clamp(k+off,0,6) is computed
   on-chip, scattered ONCE into per-tile-contiguous DRAM rows; each tile
   loads its 4KB row and expands it across partitions with the GPSIMD
   partition_broadcast instruction (no 512KB broadcast DMAs), then
   then sub/|x| on DVE, relu on ACT; Pool runs ONLY partition_broadcast
   so exactly one GPSIMD ucode library is ever loaded (a Pool tensor op
   from another library would force a ~12us library swap per tile);
 - interpolation weights of each (o,k) sum to exactly 1, so bias is
   folded into the k=0 Y eviction (tensor_tensor add of a pre-broadcast
   bias tile) — no bias matmul;
 - diag ships as bf16 (exact for ints in [-120,127]), cast once on ACT;
   output is stored bf16 and cast/transposed on the host.
Sharding: 8 cores = 4 batches x 2 halves of out_len (data parallel, no comm).
"""

import os
import sys

import ml_dtypes
import numpy as np

for _p in ("/opt/trn_rl_repo", os.path.expanduser("~/.axon_site/_ro/trn_rl_repo")):
    if os.path.isdir(_p) and _p not in sys.path:
        sys.path.insert(0, _p)

import concourse.mybir as mybir
import concourse.tile as tile
from concourse import bacc
from concourse.bass_utils import run_bass_kernel_spmd

B, CIN, COUT, L, K = 4, 512, 512, 4096, 7
OUT_LEN = 4090
HALF = 2045          # out positions per core (2 halves per batch)
OT = 121             # o-tile size -> tap band = OT + 7 = 128
TILES = 17           # 17 * 121 = 2057 >= 2045
OPAD = TILES * OT    # 2057 padded out positions per core
XW = (TILES - 1) * OT + 128  # 2064: rightmost x column any tile reads
P = 128
NCK = CIN // P       # 4 c-chunks
XCH = ((0, 256), (256, 644), (644, 1368), (1368, XW))  # x chunk boundaries
F32 = mybir.dt.float32
BF16 = mybir.dt.bfloat16

_prog_cache = {}


def _build_program():
    nc = bacc.Bacc("TRN2", target_bir_lowering=False, debug=False)

    xs_ds = [
        nc.dram_tensor(f"xs{i}", [P, NCK, hi - lo], BF16, kind="ExternalInput")
        for i, (lo, hi) in enumerate(XCH)
    ]
    wt_d = nc.dram_tensor("wt", [P, K, NCK, COUT], BF16, kind="ExternalInput")
    offsT_d = nc.dram_tensor("offsT", [K, OPAD], F32, kind="ExternalInput")
    biasf_d = nc.dram_tensor("biasf", [1, COUT], F32, kind="ExternalInput")
    diag_d = nc.dram_tensor("diagb", [P, 1024], BF16, kind="ExternalInput")
    kcol_d = nc.dram_tensor("kcol", [K, 1], F32, kind="ExternalInput")
    out_d = nc.dram_tensor("out", [OPAD, COUT], BF16, kind="ExternalOutput")

    with tile.TileContext(nc) as tc:
        with (
            tc.tile_pool(name="const", bufs=1) as cpool,
            tc.tile_pool(name="cdram", bufs=1, space="DRAM") as dpool,
            tc.tile_pool(name="cbt", bufs=4) as cbpool,
            tc.tile_pool(name="crows", bufs=4) as crpool,
            tc.tile_pool(name="stiles", bufs=4) as stpool,
            tc.tile_pool(name="ytiles", bufs=3) as ypool,
            tc.tile_pool(name="otiles", bufs=3) as opool,
            tc.tile_pool(name="psy", bufs=5, space="PSUM") as psy,
            tc.tile_pool(name="pso", bufs=3, space="PSUM") as pso,
        ):
            # ---- tiny constants FIRST on the sync ring, ahead of bulk ----
            kcol = cpool.tile([K, 1], F32)
            nc.sync.dma_start(kcol[:], kcol_d[:])
            offsT = cpool.tile([K, OPAD], F32)
            nc.sync.dma_start(offsT[:], offsT_d[:])
            biasf = cpool.tile([1, COUT], F32)
            nc.sync.dma_start(biasf[:], biasf_d[:])

            # ---- bulk inputs split across the three DMA-capable rings
            # (sync / scalar / gpsimd), each ring's chunks in PE consumption
            # order (per-ring transfers serialize at ~110GB/s; rings overlap)
            xs = cpool.tile([P, NCK, XW], BF16)
            wt = cpool.tile([P, K, NCK, COUT], BF16)
            diagb = cpool.tile([P, 1024], BF16)

            def xs_load(ring, i):
                lo, hi = XCH[i]
                ring.dma_start(xs[:, :, lo:hi], xs_ds[i][:])

            # gpsimd ring: the tiles-0/1 x band first (before the c scatters
            # above get queued -- emitted earlier, so place xs0 here is fine)
            xs_load(nc.gpsimd, 0)
            # sync ring: first halves of every k, then late x chunks
            for k in range(K):
                nc.sync.dma_start(wt[:, k, 0:2], wt_d[:, k, 0:2])
            xs_load(nc.sync, 2)
            xs_load(nc.sync, 3)
            # scalar ring: second halves of every k (+diag after k1), then x
            nc.scalar.dma_start(wt[:, 0, 2:4], wt_d[:, 0, 2:4])
            nc.scalar.dma_start(wt[:, 1, 2:4], wt_d[:, 1, 2:4])
            nc.scalar.dma_start(diagb[:], diag_d[:])
            for k in (2, 3, 4, 5, 6):
                nc.scalar.dma_start(wt[:, k, 2:4], wt_d[:, k, 2:4])
            xs_load(nc.scalar, 1)


            # c[k, o] = clamp(k + off[k, o], 0, 6) on-chip (7 partitions)
            cexp = cpool.tile([K, OPAD], F32)
            nc.vector.tensor_scalar(
                cexp[:], offsT[:], kcol[:], 0.0,
                mybir.AluOpType.add, mybir.AluOpType.max,
            )
            nc.vector.tensor_scalar(
                cexp[:], cexp[:], 6.0, None, mybir.AluOpType.min,
            )
            # scatter c ONCE into per-tile-contiguous rows: row t holds the
            # 4 k<4 blocks at 0 and the 3 k>=4 blocks at 512, each OT wide,
            # so a single 4KB partition_broadcast serves each tile.
            c_dram2 = dpool.tile([TILES, 1024], F32)
            nc.gpsimd.dma_start(
                c_dram2[:, 0 : 4 * OT].rearrange("t (k j) -> k t j", j=OT),
                cexp[0:4, :].rearrange("k (t j) -> k t j", j=OT),
            )
            nc.gpsimd.dma_start(
                c_dram2[:, 512 : 512 + 3 * OT].rearrange("t (k j) -> k t j", j=OT),
                cexp[4:7, :].rearrange("k (t j) -> k t j", j=OT),
            )
            # bias broadcast across partitions (for the k=0 eviction fold),
            # expanded on-chip from the already-resident biasf row
            bias_bc = cpool.tile([P, COUT], F32)
            nc.gpsimd.partition_broadcast(bias_bc[:], biasf[:])

            # diag bf16 -> f32 once, on ACT (its queue is idle early)
            diag7 = cpool.tile([P, 1024], F32)
            nc.scalar.copy(diag7[:], diagb[:])

            reg = slice(0, 512 + 3 * OT)

            def s_build(t):
                """load 4KB c-row, expand across partitions ON-CHIP, then
                s = relu(1 - |c - diag|): pbcast+sub on Pool, |x| on DVE,
                relu on ACT - no 512KB broadcast DMAs, no queue hogging."""
                crow = crpool.tile([1, 1024], F32, tag="crow", name=f"cr{t}")
                cb = cbpool.tile([P, 1024], F32, tag="cb", name=f"cb{t}")
                s_sb = stpool.tile([P, 1024], BF16, tag="s_sb", name=f"s{t}")
                nc.gpsimd.dma_start(crow[:], c_dram2[t : t + 1, :])
                nc.gpsimd.partition_broadcast(cb[:, reg], crow[:, reg])
                nc.vector.tensor_tensor(
                    cb[:, reg], cb[:, reg], diag7[:, reg],
                    mybir.AluOpType.subtract,
                )
                nc.vector.scalar_tensor_tensor(
                    cb[:, reg], cb[:, reg], -1.0, cb[:, reg],
                    mybir.AluOpType.mult, mybir.AluOpType.max,
                )
                nc.scalar.activation(
                    s_sb[:, reg], cb[:, reg],
                    mybir.ActivationFunctionType.Relu,
                    bias=1.0, scale=-1.0,
                )
                return s_sb

            def y_group(y_sb, o0, k):
                """Y_k for band [o0, o0+128) -> y_sb[:, k, :], bias on k=0."""
                yp = psy.tile([P, COUT], F32, tag="yp", name="yp")
                for ci in range(NCK):
                    nc.tensor.matmul(
                        yp[:],
                        xs[:, ci, o0 : o0 + P],
                        wt[:, k, ci, :],
                        start=(ci == 0), stop=(ci == NCK - 1),
                    )
                if k == 0:
                    # S_k columns sum to exactly 1 => folding bias into Y_0
                    # adds bias[d] to out[o, d] once
                    nc.vector.tensor_tensor(
                        y_sb[:, 0, :], yp[:], bias_bc[:], mybir.AluOpType.add,
                    )
                elif k < 3:
                    nc.vector.tensor_copy(y_sb[:, k, :], yp[:])
                else:
                    nc.scalar.copy(y_sb[:, k, :], yp[:])

            def stage2(s_sb, y_sb, o0, t):
                """out[o, d] = sum_k S_k^T Y_k, evict bf16, store."""
                op = pso.tile([P, COUT], F32, tag="op", name="op")
                for k in range(K):
                    koff = k * OT if k < 4 else 512 + (k - 4) * OT
                    nc.tensor.matmul(
                        op[:OT],
                        s_sb[:, koff : koff + OT],
                        y_sb[:, k, :],
                        start=(k == 0), stop=(k == K - 1),
                    )
                o_sb = opool.tile([P, COUT], BF16, tag="o_sb", name="o_sb")
                if t % 2 == 0:
                    nc.scalar.copy(o_sb[:OT], op[:OT])
                else:
                    nc.vector.tensor_copy(o_sb[:OT], op[:OT])
                nc.sync.dma_start(out_d[o0 : o0 + OT, :], o_sb[:OT])

            # ---- tiles 0,1: stage-1 k-loops interleaved so PE consumption
            # (2 x 4 matmuls per k) rides the per-k weight arrival rate
            s0, s1 = s_build(0), s_build(1)
            y0 = ypool.tile([P, K, COUT], BF16, tag="y_sb", name="y0")
            y1 = ypool.tile([P, K, COUT], BF16, tag="y_sb", name="y1")
            for k in range(K):
                y_group(y0, 0, k)
                y_group(y1, OT, k)
            stage2(s0, y0, 0, 0)
            stage2(s1, y1, OT, 1)

            # ---- tiles 2..16, steady state ----
            for t in range(2, TILES):
                o0 = t * OT
                s_sb = s_build(t)
                y_sb = ypool.tile([P, K, COUT], BF16, tag="y_sb", name="y_sb")
                for k in range(K):
                    y_group(y_sb, o0, k)
                stage2(s_sb, y_sb, o0, t)

    nc.compile()
    return nc


def _install_axon_ntff_hook():
    """Provide antenv.axon_hooks (absent on this image) so that
    run_bass_kernel_spmd(trace=True) can capture NTFF profiles via the
    axon .so's C ABI.  Mirrors trn_agent_boot.trn_boot."""
    import contextlib
    import ctypes
    import types

    try:
        from antenv.axon_hooks import set_axon_ntff_profile_hook  # noqa: F401
        return
    except ImportError:
        pass

    so_path = "/opt/axon/libaxon_pjrt.so"
    if not os.path.exists(so_path):
        return
    lib = ctypes.CDLL(so_path)
    if not hasattr(lib, "axon_start_nrt_profile"):
        return
    lib.axon_start_nrt_profile.argtypes = [
        ctypes.POINTER(ctypes.c_int64), ctypes.c_size_t,
    ]
    lib.axon_start_nrt_profile.restype = ctypes.c_int64
    lib.axon_stop_nrt_profile.argtypes = [ctypes.c_char_p]
    lib.axon_stop_nrt_profile.restype = ctypes.c_int64

    @contextlib.contextmanager
    def _hook(output_dir, device_ids):
        import jax

        jax.devices()
        if device_ids:
            ids = (ctypes.c_int64 * len(device_ids))(*device_ids)
            rc = lib.axon_start_nrt_profile(ids, len(device_ids))
        else:
            rc = lib.axon_start_nrt_profile(None, 0)
        if rc != 0:
            raise RuntimeError(f"axon_start_nrt_profile rc={rc}")
        try:
            yield
        finally:
            n = lib.axon_stop_nrt_profile(str(output_dir).encode())
            print(f"ntff profile: {n} file(s) written to {output_dir}")

    box = {"h": _hook}
    mod = types.ModuleType("antenv.axon_hooks")
    mod.get_axon_ntff_profile_hook = lambda: box["h"]
    mod.set_axon_ntff_profile_hook = lambda h: box.__setitem__("h", h)
    import antenv

    sys.modules["antenv.axon_hooks"] = mod
    antenv.axon_hooks = mod

    # zero-egress env: skip the artifact upload in the trace path
    from concourse import bass_utils as _bu

    _bu.upload_artifacts = lambda d: f"local:{d}"


def _consts():
    # diag7[dp, koff(k)+j] = dp - j   (j = o - o0), bank-padded layout:
    # k<4 at k*OT, k>=4 at 512+(k-4)*OT; values in [-120, 127] are exact bf16
    dp = np.arange(P, dtype=np.float32).reshape(P, 1)
    j = np.arange(OT, dtype=np.float32).reshape(1, OT)
    blk = dp - j  # [P, OT]
    diag7 = np.zeros((P, 1024), dtype=np.float32)
    for i in range(4):
        diag7[:, i * OT : i * OT + OT] = blk
        diag7[:, 512 + i * OT : 512 + i * OT + OT] = blk
    kcol = np.arange(K, dtype=np.float32).reshape(K, 1).copy()
    return diag7, kcol


def kernel(x, offsets, weight, bias, _trace=False, _trace_kwargs=None):
    x = np.asarray(x, dtype=np.float32)
    offsets = np.asarray(offsets, dtype=np.float32)
    weight = np.asarray(weight, dtype=np.float32)
    bias = np.asarray(bias, dtype=np.float32)

    if "nc" not in _prog_cache:
        _prog_cache["nc"] = _build_program()
    nc = _prog_cache["nc"]

    # W packed [p, k, ci, d] so each per-k DMA is contiguous per partition
    w4 = np.ascontiguousarray(
        np.transpose(weight, (1, 2, 0))              # [c, k, d]
        .reshape(NCK, P, K, COUT)
        .transpose(1, 2, 0, 3)                       # [p, k, ci, d]
        .astype(ml_dtypes.bfloat16)
    )
    biasf = np.ascontiguousarray(bias.reshape(1, COUT).astype(np.float32))
    diag7, kcol = _consts()
    diagb = diag7.astype(ml_dtypes.bfloat16)

    in_maps = []
    for core in range(8):
        b, half = core // 2, core % 2
        o_off = half * HALF
        xsf = np.zeros((P, NCK, XW), dtype=ml_dtypes.bfloat16)
        xw = min(L - o_off, XW)
        # x rows c = ci*128 + p -> [p, ci, t]
        xsf[:, :, :xw] = (
            x[b][:, o_off : o_off + xw]
            .reshape(NCK, P, xw)
            .transpose(1, 0, 2)
            .astype(ml_dtypes.bfloat16)
        )
        offsT = np.zeros((K, OPAD), dtype=np.float32)
        ow = min(OUT_LEN - o_off, OPAD)
        offsT[:, :ow] = offsets[b, 0, o_off : o_off + ow, :].T
        im = {
            "wt": w4, "offsT": offsT, "biasf": biasf,
            "diagb": diagb, "kcol": kcol,
        }
        for i, (lo, hi) in enumerate(XCH):
            im[f"xs{i}"] = np.ascontiguousarray(xsf[:, :, lo:hi])
        in_maps.append(im)

    if _trace:
        _install_axon_ntff_hook()
    try:
        res = run_bass_kernel_spmd(
            nc, in_maps, core_ids=list(range(8)),
            trace=_trace, **(_trace_kwargs or {}),
        )
    except Exception:
        # transient runtime faults have been observed; one retry
        res = run_bass_kernel_spmd(
            nc, in_maps, core_ids=list(range(8)),
            trace=_trace, **(_trace_kwargs or {}),
        )

    out = np.empty((B, COUT, OUT_LEN), dtype=np.float32)
    for core in range(8):
        b, half = core // 2, core % 2
        o_off = half * HALF
        out[b, :, o_off : o_off + HALF] = (
            res.results[core]["out"][:HALF, :].astype(np.float32).T
        )
    if _trace:
        _prog_cache["last_exec_time_ns"] = res.exec_time_ns
    return out
